# revision 15
# baseline (speedup 1.0000x reference)
"""Trainium2 Bass kernel for nn_Attention_13039520711118 (attention pooling).

reference:
    h = hidden[:, -1, :]
    m = enc @ M_w[:, :E].T + h @ M_w[:, E:].T + M_b        # (B, S, H)
    scores = tanh(m) @ V_w[0] + V_b                        # (B, S)
    scores = where(mask, -1e9, scores)
    weights = softmax(scores, axis=1)[:, None, :]          # (B, 1, S)
    weighted = weights @ enc                               # (B, 1, E)
    return weighted, weights

Sharding: data-parallel over batch B=16 across 8 cores (2 batches/core);
M_w / M_b / V_w replicated.

Per-core pipeline (all shapes hardcoded):
  mm1 in bf16: cast-load encoded -> PE-transpose 128x128 tiles -> encT,
  mT[h,s] = sum_e M_eT[e,h].T @ encT[e,s] (PSUM f32), tanh(+per-h bias) on ACT,
  scores = V.T @ tanh on PE (M=1 matmuls, PSUM accumulate).
  Bias = M_h @ h + M_b via DVE tensor_tensor_reduce against a partition-
  broadcast copy of h.
  Softmax in f32 on DVE/ACT (V_b dropped: softmax is shift-invariant; masked
  entries round to exactly -1e9 in f32, matching the reference's fill).
  mm2 in float32r (~1e-4 rel err at full PE rate): weighted = wT.T @ enc,
  streaming a second fp32 read of encoded.
"""
import sys

sys.path.insert(0, "/opt/trn_rl_repo")

from contextlib import ExitStack

import numpy as np

import concourse.bacc as bacc
import concourse.bass as bass
import concourse.mybir as mybir
import concourse.tile as tile
from concourse import masks
from concourse.bass_utils import run_bass_kernel_spmd

F32 = mybir.dt.float32
F32R = mybir.dt.float32r
BF16 = mybir.dt.bfloat16
U8 = mybir.dt.uint8
AF = mybir.ActivationFunctionType
ALU = mybir.AluOpType
AX = mybir.AxisListType

N_CORES = 8
B, S, E, H = 16, 2048, 2048, 1024
BPC = B // N_CORES          # batches per core
SC = 512                    # s-chunk (columns per mm1 matmul)
NSC = S // SC               # 4 s-chunks per batch
NET = E // 128              # 16 e-tiles
NHT = H // 128              # 8 h-tiles
NST = S // 128              # 16 s-tiles
HG = 2                      # h-tiles per psum group
NEG = -1e9

LAST_EXEC_NS = None         # set by test harness runs with trace=True


def _build():
    nc = bacc.Bacc("TRN2", target_bir_lowering=False, debug=False,
                   num_devices=N_CORES)

    enc_d = nc.dram_tensor("enc", [BPC, S, E], F32, kind="ExternalInput")
    hid_d = nc.dram_tensor("hid", [BPC, H], F32, kind="ExternalInput")
    mask_d = nc.dram_tensor("mask", [BPC, S], U8, kind="ExternalInput")
    mw_d = nc.dram_tensor("mw", [H, E + H], F32, kind="ExternalInput")
    mbT_d = nc.dram_tensor("mbT", [128, NHT], F32, kind="ExternalInput")
    vT_d = nc.dram_tensor("vT", [128, NHT], F32, kind="ExternalInput")

    w_o = nc.dram_tensor("w_o", [BPC, S], F32, kind="ExternalOutput")
    ws_o = nc.dram_tensor("ws_o", [BPC, E], F32, kind="ExternalOutput")

    with tile.TileContext(nc) as tc, ExitStack() as ctx:
        const = ctx.enter_context(tc.tile_pool(name="const", bufs=1))
        meT_p = ctx.enter_context(tc.tile_pool(name="meT", bufs=NET))
        nat_p = ctx.enter_context(tc.tile_pool(name="nat", bufs=8))
        e512_p = ctx.enter_context(tc.tile_pool(name="e512", bufs=32))
        tanh_p = ctx.enter_context(tc.tile_pool(name="tanh", bufs=10))
        f32s_p = ctx.enter_context(tc.tile_pool(name="f32s", bufs=3))
        vec_p = ctx.enter_context(tc.tile_pool(name="vec", bufs=4))
        small_p = ctx.enter_context(tc.tile_pool(name="small", bufs=2))
        acc_p = ctx.enter_context(tc.tile_pool(name="acc", bufs=4, space="PSUM"))
        aux_p = ctx.enter_context(tc.tile_pool(name="aux", bufs=3, space="PSUM"))

        # ---------------- constants / small setup ----------------
        ident = const.tile([128, 128], BF16)
        masks.make_identity(nc, ident[:])
        one1 = const.tile([1, 1], F32)
        nc.gpsimd.memset(one1[:], 1.0)

        vT = const.tile([128, NHT], BF16)
        nc.gpsimd.dma_start(vT[:], vT_d[:, :])          # cast f32 -> bf16
        mbT = const.tile([128, NHT], F32)
        nc.sync.dma_start(mbT[:], mbT_d[:, :])

        mask_sb = []
        for b in range(BPC):
            t = const.tile([1, S], U8, name=f"mask{b}")
            nc.sync.dma_start(t[:], mask_d[b:b + 1, :])
            mask_sb.append(t)

        # h broadcast across partitions: [128, H] per batch (f32)
        hbc = []
        for b in range(BPC):
            t = const.tile([128, H], F32, name=f"hbc{b}")
            nc.sync.dma_start(t[:], bass.AP(hid_d, b * H, [[0, 128], [1, H]]))
            hbc.append(t)

        # bias[h, b] = M_h @ h_b + M_b via DVE multiply + reduce (bf16)
        bias_sb = const.tile([128, NHT * BPC], F32)     # col = ht*BPC + b
        hbc16 = []
        for b in range(BPC):
            t16 = const.tile([128, H], BF16, name=f"hbc16_{b}")
            nc.scalar.copy(t16[:], hbc[b][:])
            hbc16.append(t16)
        mh_tiles = {}
        for ht in range(NHT):
            for dh in range(2):                          # H = 2 x 512
                t = e512_p.tile([128, 512], BF16, tag="e512")
                nc.gpsimd.dma_start(
                    t[:], mw_d[ht * 128:(ht + 1) * 128,
                               E + dh * 512:E + (dh + 1) * 512])
                mh_tiles[(ht, dh)] = t
        for b in range(BPC):
            for ht in range(NHT):
                col = ht * BPC + b
                pp = small_p.tile([128, 2], F32, tag="pp", name=f"pp{b}_{ht}")
                for dh in range(2):
                    scr = small_p.tile([128, 512], BF16, tag="ttrs",
                                       name=f"scr{b}_{ht}_{dh}")
                    nc.vector.tensor_mul(
                        scr[:], mh_tiles[(ht, dh)][:],
                        hbc16[b][:, dh * 512:(dh + 1) * 512])
                    nc.vector.tensor_reduce(pp[:, dh:dh + 1], scr[:],
                                            axis=AX.X, op=ALU.add)
                nc.vector.tensor_add(pp[:, 0:1], pp[:, 0:1], pp[:, 1:2])
                nc.vector.tensor_add(bias_sb[:, col:col + 1], pp[:, 0:1],
                                     mbT[:, ht:ht + 1])

        # ---------------- M_eT: [e, h] bf16, 16 tiles of [128, H] ----------
        meT = []
        me_nat = []
        for ht in range(NHT):
            t = nat_p.tile([128, E], BF16, tag="nat")
            nc.gpsimd.dma_start(t[:], mw_d[ht * 128:(ht + 1) * 128, 0:E])
            me_nat.append(t)
        for et in range(NET):
            mt = meT_p.tile([128, H], BF16, tag="meT")
            for hh in range(0, NHT, 4):                 # pack 4 h-tiles per bank
                pt = aux_p.tile([128, 512], BF16, tag="aux")
                for ht in range(hh, hh + 4):
                    nc.tensor.transpose(
                        pt[:, (ht - hh) * 128:(ht - hh + 1) * 128],
                        me_nat[ht][:, et * 128:(et + 1) * 128], ident[:])
                nc.vector.tensor_copy(mt[:, hh * 128:(hh + 4) * 128], pt[:])
            meT.append(mt)

        # ---------------- main per-batch pipeline ----------------
        scores_sb = {}

        def mm1(b):
            sc_chunks = []
            nat = []
            for st in range(NST):
                t = nat_p.tile([128, E], BF16, tag="nat")
                nc.gpsimd.dma_start(t[:], enc_d[b, st * 128:(st + 1) * 128, :])
                nat.append(t)
            for sc in range(NSC):
                # transpose the 4 s-tiles of this chunk into encT tiles
                encT = []
                for et in range(NET):
                    pt = aux_p.tile([128, SC], BF16, tag="aux")
                    for j in range(SC // 128):
                        st = sc * (SC // 128) + j
                        nc.tensor.transpose(
                            pt[:, j * 128:(j + 1) * 128],
                            nat[st][:, et * 128:(et + 1) * 128], ident[:])
                    t = e512_p.tile([128, SC], BF16, tag="e512")
                    nc.vector.tensor_copy(t[:], pt[:])
                    encT.append(t)
                # matmuls + tanh, h-groups of HG
                sc_ps = aux_p.tile([1, SC], F32, tag="aux")
                tanh_tiles = []
                for hg in range(NHT // HG):
                    accs = [acc_p.tile([128, SC], F32, tag="acc", name=f"acc{hh}")
                            for hh in range(HG)]
                    for et in range(NET):
                        for hh in range(HG):
                            ht = hg * HG + hh
                            nc.tensor.matmul(
                                accs[hh][:, :],
                                meT[et][:, ht * 128:(ht + 1) * 128],
                                encT[et][:, :],
                                start=(et == 0), stop=(et == NET - 1))
                    for hh in range(HG):
                        ht = hg * HG + hh
                        tt = tanh_p.tile([128, SC], BF16, tag="tanh")
                        nc.scalar.activation(
                            tt[:], accs[hh][:], AF.Tanh,
                            bias=bias_sb[:, ht * BPC + b:ht * BPC + b + 1])
                        tanh_tiles.append(tt)
                # V-dot: scores[1, SC] accumulated over all 8 h-tiles
                for ht in range(NHT):
                    nc.tensor.matmul(sc_ps[:, :], vT[:, ht:ht + 1],
                                     tanh_tiles[ht][:, :],
                                     start=(ht == 0), stop=(ht == NHT - 1))
                sc_chunks.append(sc_ps)
            ssb = vec_p.tile([1, S], F32, tag="vec", name=f"ssb{b}")
            for sc in range(NSC):
                nc.vector.tensor_copy(ssb[:, sc * SC:(sc + 1) * SC],
                                      sc_chunks[sc][:])
            scores_sb[b] = ssb

        def softmax(b):
            ssb = scores_sb[b]
            mneg = vec_p.tile([1, S], F32, tag="vec", name=f"mneg{b}")
            nc.vector.tensor_scalar_mul(mneg[:], mask_sb[b][:], NEG)
            nc.vector.tensor_add(ssb[:], ssb[:], mneg[:])
            negmx = small_p.tile([1, 1], F32, tag="negmx")
            nc.vector.tensor_reduce(negmx[:], ssb[:], axis=AX.X,
                                    op=ALU.max, negate=True)
            expv = vec_p.tile([1, S], F32, tag="vec", name=f"expv{b}")
            zs = small_p.tile([1, 1], F32, tag="zs")
            nc.scalar.activation(expv[:], ssb[:], AF.Exp,
                                 bias=negmx[:, 0:1], accum_out=zs[:, 0:1])
            rz = small_p.tile([1, 1], F32, tag="rz")
            nc.vector.reciprocal(rz[:], zs[:])
            w_sb = vec_p.tile([1, S], F32, tag="vec", name=f"wsb{b}")
            nc.vector.tensor_scalar_mul(w_sb[:], expv[:], rz[:, 0:1])
            nc.sync.dma_start(w_o[b:b + 1, :], w_sb[:])
            return w_sb

        def mm2(b, w_sb):
            # wT[s, 1] per s-tile via PE transpose
            wtp = aux_p.tile([128, NST], F32, tag="aux")
            for st in range(NST):
                nc.tensor.transpose(wtp[:, st:st + 1],
                                    w_sb[0:1, st * 128:(st + 1) * 128],
                                    one1[:])
            wT = small_p.tile([128, NST], F32R, tag="wT")
            nc.vector.tensor_copy(wT[:], wtp[:])
            wTr = wT[:]

            waccs = [acc_p.tile([1, 512], F32, tag="acc", name=f"wacc{ec}")
                     for ec in range(4)]
            for st in range(NST):
                t = f32s_p.tile([128, E], F32R, tag="f32s")
                nc.gpsimd.dma_start(t[:], enc_d[b, st * 128:(st + 1) * 128, :])
                tr = t[:]
                for ec in range(4):
                    nc.tensor.matmul(waccs[ec][:, :], wTr[:, st:st + 1],
                                     tr[:, ec * 512:(ec + 1) * 512],
                                     start=(st == 0), stop=(st == NST - 1))
            ws_sb = vec_p.tile([1, E], F32, tag="vec", name=f"wssb{b}")
            for ec in range(4):
                nc.vector.tensor_copy(ws_sb[:, ec * 512:(ec + 1) * 512],
                                      waccs[ec][:])
            nc.sync.dma_start(ws_o[b:b + 1, :], ws_sb[:])

        mm1(0)
        w0 = softmax(0)
        mm1(1)
        mm2(0, w0)
        w1 = softmax(1)
        mm2(1, w1)

    nc.compile()
    return nc


_NC = None


def _get_nc():
    global _NC
    if _NC is None:
        _NC = _build()
    return _NC


def kernel(encoded, hidden, mask, M_w, M_b, V_w, V_b, _trace=False):
    global LAST_EXEC_NS
    encoded = np.ascontiguousarray(np.asarray(encoded, dtype=np.float32))
    hidden = np.asarray(hidden, dtype=np.float32)
    mask_u8 = np.asarray(mask).astype(np.uint8)
    M_w = np.ascontiguousarray(np.asarray(M_w, dtype=np.float32))
    M_b = np.asarray(M_b, dtype=np.float32)
    V_w = np.asarray(V_w, dtype=np.float32)
    # V_b is unused: softmax(s + c) == softmax(s), and masked entries are
    # exactly -1e9 with or without it.

    mbT = np.ascontiguousarray(M_b.reshape(NHT, 128).T)          # [128, 8]
    vT = np.ascontiguousarray(V_w[0].reshape(NHT, 128).T)        # [128, 8]
    hid2 = np.ascontiguousarray(hidden[:, -1, :])                # [B, H]

    nc = _get_nc()
    in_maps = []
    for c in range(N_CORES):
        sl = slice(c * BPC, (c + 1) * BPC)
        in_maps.append({
            "enc": encoded[sl],
            "hid": np.ascontiguousarray(hid2[sl]),
            "mask": np.ascontiguousarray(mask_u8[sl]),
            "mw": M_w,
            "mbT": mbT,
            "vT": vT,
        })

    res = run_bass_kernel_spmd(nc, in_maps, core_ids=list(range(N_CORES)),
                               trace=_trace)
    LAST_EXEC_NS = res.exec_time_ns

    weights = np.concatenate([r["w_o"] for r in res.results], axis=0)
    weighted = np.concatenate([r["ws_o"] for r in res.results], axis=0)
    return weighted[:, None, :].astype(np.float32), \
        weights[:, None, :].astype(np.float32)


# revision 17
# speedup vs baseline: 1.0144x; 1.0144x over previous
"""Trainium2 Bass kernel for nn_Attention_13039520711118 (attention pooling).

reference:
    h = hidden[:, -1, :]
    m = enc @ M_w[:, :E].T + h @ M_w[:, E:].T + M_b        # (B, S, H)
    scores = tanh(m) @ V_w[0] + V_b                        # (B, S)
    scores = where(mask, -1e9, scores)
    weights = softmax(scores, axis=1)[:, None, :]          # (B, 1, S)
    weighted = weights @ enc                               # (B, 1, E)
    return weighted, weights

Sharding: data-parallel over batch B=16 across 8 cores (2 batches/core);
M_w / M_b / V_w replicated.

Per-core pipeline (all shapes hardcoded):
  mm1 in bf16: cast-load encoded -> PE-transpose 128x128 tiles -> encT,
  mT[h,s] = sum_e M_eT[e,h].T @ encT[e,s] (PSUM f32), tanh(+per-h bias) on ACT,
  scores = V.T @ tanh on PE (M=1 matmuls, PSUM accumulate).
  Bias = M_h @ h + M_b via DVE tensor_tensor_reduce against a partition-
  broadcast copy of h.
  Softmax in f32 on DVE/ACT (V_b dropped: softmax is shift-invariant; masked
  entries round to exactly -1e9 in f32, matching the reference's fill).
  mm2 in float32r (~1e-4 rel err at full PE rate): weighted = wT.T @ enc,
  streaming a second fp32 read of encoded.
"""
import sys

sys.path.insert(0, "/opt/trn_rl_repo")

from contextlib import ExitStack

import numpy as np

import concourse.bacc as bacc
import concourse.bass as bass
import concourse.mybir as mybir
import concourse.tile as tile
from concourse import masks
from concourse.bass_utils import run_bass_kernel_spmd

F32 = mybir.dt.float32
F32R = mybir.dt.float32r
BF16 = mybir.dt.bfloat16
U8 = mybir.dt.uint8
AF = mybir.ActivationFunctionType
ALU = mybir.AluOpType
AX = mybir.AxisListType

N_CORES = 8
B, S, E, H = 16, 2048, 2048, 1024
BPC = B // N_CORES          # batches per core
SC = 512                    # s-chunk (columns per mm1 matmul)
NSC = S // SC               # 4 s-chunks per batch
NET = E // 128              # 16 e-tiles
NHT = H // 128              # 8 h-tiles
NST = S // 128              # 16 s-tiles
HG = 2                      # h-tiles per psum group
NEG = -1e9

LAST_EXEC_NS = None         # set by test harness runs with trace=True


def _build():
    nc = bacc.Bacc("TRN2", target_bir_lowering=False, debug=False,
                   num_devices=N_CORES)

    enc_d = nc.dram_tensor("enc", [BPC, S, E], F32, kind="ExternalInput")
    hid_d = nc.dram_tensor("hid", [BPC, H], F32, kind="ExternalInput")
    mask_d = nc.dram_tensor("mask", [BPC, S], U8, kind="ExternalInput")
    mw_d = nc.dram_tensor("mw", [H, E + H], F32, kind="ExternalInput")
    mbT_d = nc.dram_tensor("mbT", [128, NHT], F32, kind="ExternalInput")
    vT_d = nc.dram_tensor("vT", [128, NHT], F32, kind="ExternalInput")

    w_o = nc.dram_tensor("w_o", [BPC, S], F32, kind="ExternalOutput")
    ws_o = nc.dram_tensor("ws_o", [BPC, E], F32, kind="ExternalOutput")

    with tile.TileContext(nc) as tc, ExitStack() as ctx:
        const = ctx.enter_context(tc.tile_pool(name="const", bufs=1))
        meT_p = ctx.enter_context(tc.tile_pool(name="meT", bufs=NET))
        nat_p = ctx.enter_context(tc.tile_pool(name="nat", bufs=8))
        e512_p = ctx.enter_context(tc.tile_pool(name="e512", bufs=32))
        tanh_p = ctx.enter_context(tc.tile_pool(name="tanh", bufs=10))
        f32s_p = ctx.enter_context(tc.tile_pool(name="f32s", bufs=3))
        vec_p = ctx.enter_context(tc.tile_pool(name="vec", bufs=4))
        small_p = ctx.enter_context(tc.tile_pool(name="small", bufs=2))
        acc_p = ctx.enter_context(tc.tile_pool(name="acc", bufs=4, space="PSUM"))
        aux_p = ctx.enter_context(tc.tile_pool(name="aux", bufs=3, space="PSUM"))

        # ---------------- constants / small setup ----------------
        ident = const.tile([128, 128], BF16)
        masks.make_identity(nc, ident[:])
        one1 = const.tile([1, 1], F32)
        nc.gpsimd.memset(one1[:], 1.0)

        vT = const.tile([128, NHT], BF16)
        nc.gpsimd.dma_start(vT[:], vT_d[:, :])          # cast f32 -> bf16
        mbT = const.tile([128, NHT], F32)
        nc.sync.dma_start(mbT[:], mbT_d[:, :])

        mask_sb = []
        for b in range(BPC):
            t = const.tile([1, S], U8, name=f"mask{b}")
            nc.sync.dma_start(t[:], mask_d[b:b + 1, :])
            mask_sb.append(t)

        # h broadcast across partitions: [128, H] per batch (f32)
        hbc = []
        for b in range(BPC):
            t = const.tile([128, H], F32, name=f"hbc{b}")
            nc.sync.dma_start(t[:], bass.AP(hid_d, b * H, [[0, 128], [1, H]]))
            hbc.append(t)

        # bias[h, b] = M_h @ h_b + M_b via DVE multiply + reduce (bf16)
        bias_sb = const.tile([128, NHT * BPC], F32)     # col = ht*BPC + b
        hbc16 = []
        for b in range(BPC):
            t16 = const.tile([128, H], BF16, name=f"hbc16_{b}")
            nc.scalar.copy(t16[:], hbc[b][:])
            hbc16.append(t16)
        mh_tiles = {}
        for ht in range(NHT):
            for dh in range(2):                          # H = 2 x 512
                t = e512_p.tile([128, 512], BF16, tag="e512")
                nc.gpsimd.dma_start(
                    t[:], mw_d[ht * 128:(ht + 1) * 128,
                               E + dh * 512:E + (dh + 1) * 512])
                mh_tiles[(ht, dh)] = t
        for b in range(BPC):
            for ht in range(NHT):
                col = ht * BPC + b
                pp = small_p.tile([128, 2], F32, tag="pp", name=f"pp{b}_{ht}")
                for dh in range(2):
                    scr = small_p.tile([128, 512], BF16, tag="ttrs",
                                       name=f"scr{b}_{ht}_{dh}")
                    nc.vector.tensor_mul(
                        scr[:], mh_tiles[(ht, dh)][:],
                        hbc16[b][:, dh * 512:(dh + 1) * 512])
                    nc.vector.tensor_reduce(pp[:, dh:dh + 1], scr[:],
                                            axis=AX.X, op=ALU.add)
                nc.vector.tensor_add(pp[:, 0:1], pp[:, 0:1], pp[:, 1:2])
                nc.vector.tensor_add(bias_sb[:, col:col + 1], pp[:, 0:1],
                                     mbT[:, ht:ht + 1])

        # ---------------- M_eT: [e, h] bf16, 16 tiles of [128, H] ----------
        meT = []
        me_nat = []
        for ht in range(NHT):
            t = nat_p.tile([128, E], BF16, tag="nat")
            nc.gpsimd.dma_start(t[:], mw_d[ht * 128:(ht + 1) * 128, 0:E])
            me_nat.append(t)
        for et in range(NET):
            mt = meT_p.tile([128, H], BF16, tag="meT")
            for hh in range(0, NHT, 4):                 # pack 4 h-tiles per bank
                pt = aux_p.tile([128, 512], BF16, tag="aux")
                for ht in range(hh, hh + 4):
                    nc.tensor.transpose(
                        pt[:, (ht - hh) * 128:(ht - hh + 1) * 128],
                        me_nat[ht][:, et * 128:(et + 1) * 128], ident[:])
                nc.vector.tensor_copy(mt[:, hh * 128:(hh + 4) * 128], pt[:])
            meT.append(mt)

        # ---------------- main per-batch pipeline ----------------
        scores_sb = {}

        def mm1(b):
            sc_chunks = []
            nat = []
            for st in range(NST):
                t = nat_p.tile([128, E], BF16, tag="nat")
                nc.gpsimd.dma_start(t[:], enc_d[b, st * 128:(st + 1) * 128, :])
                nat.append(t)
            for sc in range(NSC):
                # transpose the 4 s-tiles of this chunk into encT tiles
                encT = []
                for et in range(NET):
                    pt = aux_p.tile([128, SC], BF16, tag="aux")
                    for j in range(SC // 128):
                        st = sc * (SC // 128) + j
                        nc.tensor.transpose(
                            pt[:, j * 128:(j + 1) * 128],
                            nat[st][:, et * 128:(et + 1) * 128], ident[:])
                    t = e512_p.tile([128, SC], BF16, tag="e512")
                    nc.vector.tensor_copy(t[:], pt[:])
                    encT.append(t)
                # matmuls + tanh, h-groups of HG
                sc_ps = aux_p.tile([1, SC], F32, tag="aux")
                tanh_tiles = []
                for hg in range(NHT // HG):
                    accs = [acc_p.tile([128, SC], F32, tag="acc", name=f"acc{hh}")
                            for hh in range(HG)]
                    for et in range(NET):
                        for hh in range(HG):
                            ht = hg * HG + hh
                            nc.tensor.matmul(
                                accs[hh][:, :],
                                meT[et][:, ht * 128:(ht + 1) * 128],
                                encT[et][:, :],
                                start=(et == 0), stop=(et == NET - 1))
                    for hh in range(HG):
                        ht = hg * HG + hh
                        tt = tanh_p.tile([128, SC], BF16, tag="tanh")
                        nc.scalar.activation(
                            tt[:], accs[hh][:], AF.Tanh,
                            bias=bias_sb[:, ht * BPC + b:ht * BPC + b + 1])
                        tanh_tiles.append(tt)
                # V-dot: scores[1, SC] accumulated over all 8 h-tiles
                for ht in range(NHT):
                    nc.tensor.matmul(sc_ps[:, :], vT[:, ht:ht + 1],
                                     tanh_tiles[ht][:, :],
                                     start=(ht == 0), stop=(ht == NHT - 1))
                sc_chunks.append(sc_ps)
            ssb = vec_p.tile([1, S], F32, tag="vec", name=f"ssb{b}")
            for sc in range(NSC):
                nc.vector.tensor_copy(ssb[:, sc * SC:(sc + 1) * SC],
                                      sc_chunks[sc][:])
            scores_sb[b] = ssb

        def softmax(b):
            ssb = scores_sb[b]
            mneg = vec_p.tile([1, S], F32, tag="vec", name=f"mneg{b}")
            nc.vector.tensor_scalar_mul(mneg[:], mask_sb[b][:], NEG)
            nc.vector.tensor_add(ssb[:], ssb[:], mneg[:])
            negmx = small_p.tile([1, 1], F32, tag="negmx")
            nc.vector.tensor_reduce(negmx[:], ssb[:], axis=AX.X,
                                    op=ALU.max, negate=True)
            expv = vec_p.tile([1, S], F32, tag="vec", name=f"expv{b}")
            zs = small_p.tile([1, 1], F32, tag="zs")
            nc.scalar.activation(expv[:], ssb[:], AF.Exp,
                                 bias=negmx[:, 0:1], accum_out=zs[:, 0:1])
            rz = small_p.tile([1, 1], F32, tag="rz")
            nc.vector.reciprocal(rz[:], zs[:])
            w_sb = vec_p.tile([1, S], F32, tag="vec", name=f"wsb{b}")
            nc.vector.tensor_scalar_mul(w_sb[:], expv[:], rz[:, 0:1])
            nc.sync.dma_start(w_o[b:b + 1, :], w_sb[:])
            return w_sb

        def mm2(b, w_sb):
            # wT[s, 1] per s-tile via PE transpose
            wtp = aux_p.tile([128, NST], F32, tag="aux")
            for st in range(NST):
                nc.tensor.transpose(wtp[:, st:st + 1],
                                    w_sb[0:1, st * 128:(st + 1) * 128],
                                    one1[:])
            wT = small_p.tile([128, NST], F32R, tag="wT")
            nc.vector.tensor_copy(wT[:], wtp[:])
            wTr = wT[:]

            waccs = [acc_p.tile([1, 512], F32, tag="acc", name=f"wacc{ec}")
                     for ec in range(4)]
            for st in range(NST):
                t = f32s_p.tile([128, E], F32R, tag="f32s")
                nc.gpsimd.dma_start(t[:], enc_d[b, st * 128:(st + 1) * 128, :])
                tr = t[:]
                for ec in range(4):
                    nc.tensor.matmul(waccs[ec][:, :], wTr[:, st:st + 1],
                                     tr[:, ec * 512:(ec + 1) * 512],
                                     start=(st == 0), stop=(st == NST - 1))
            ws_sb = vec_p.tile([1, E], F32, tag="vec", name=f"wssb{b}")
            for ec in range(4):
                nc.vector.tensor_copy(ws_sb[:, ec * 512:(ec + 1) * 512],
                                      waccs[ec][:])
            nc.sync.dma_start(ws_o[b:b + 1, :], ws_sb[:])

        mm1(0)
        w0 = softmax(0)
        mm1(1)
        mm2(0, w0)
        w1 = softmax(1)
        mm2(1, w1)

    nc.compile()
    return nc


_NC = None


def _get_nc():
    global _NC
    if _NC is None:
        _NC = _build()
    return _NC


def kernel(encoded, hidden, mask, M_w, M_b, V_w, V_b, _trace=False,
           _tmpdir=None):
    global LAST_EXEC_NS
    encoded = np.ascontiguousarray(np.asarray(encoded, dtype=np.float32))
    hidden = np.asarray(hidden, dtype=np.float32)
    mask_u8 = np.asarray(mask).astype(np.uint8)
    M_w = np.ascontiguousarray(np.asarray(M_w, dtype=np.float32))
    M_b = np.asarray(M_b, dtype=np.float32)
    V_w = np.asarray(V_w, dtype=np.float32)
    # V_b is unused: softmax(s + c) == softmax(s), and masked entries are
    # exactly -1e9 with or without it.

    mbT = np.ascontiguousarray(M_b.reshape(NHT, 128).T)          # [128, 8]
    vT = np.ascontiguousarray(V_w[0].reshape(NHT, 128).T)        # [128, 8]
    hid2 = np.ascontiguousarray(hidden[:, -1, :])                # [B, H]

    nc = _get_nc()
    in_maps = []
    for c in range(N_CORES):
        sl = slice(c * BPC, (c + 1) * BPC)
        in_maps.append({
            "enc": encoded[sl],
            "hid": np.ascontiguousarray(hid2[sl]),
            "mask": np.ascontiguousarray(mask_u8[sl]),
            "mw": M_w,
            "mbT": mbT,
            "vT": vT,
        })

    res = run_bass_kernel_spmd(nc, in_maps, core_ids=list(range(N_CORES)),
                               trace=_trace, tmpdir=_tmpdir)
    LAST_EXEC_NS = res.exec_time_ns

    weights = np.concatenate([r["w_o"] for r in res.results], axis=0)
    weighted = np.concatenate([r["ws_o"] for r in res.results], axis=0)
    return weighted[:, None, :].astype(np.float32), \
        weights[:, None, :].astype(np.float32)
